# revision 1
# baseline (speedup 1.0000x reference)
"""CRF Viterbi decode kernel for Trainium2 (8 NeuronCores, data-parallel over batch).

Problem: emissions [70, 32768, 37] fp32, mask [70, 32768] (all ones),
start/end transitions [37], transitions [37, 37].
Output: best tag sequence per batch element, [32768, 70] int32.

Strategy per core (B_core = 4096 = 32 partition-tiles of 128 batch rows):
  Forward max-plus scan with batch on partitions and the (j_next, i_prev)
  tag-pair expansion (37*37 = 1369) on the free dim. Exact fp32 semantics,
  including the reference's associativity  w = (score + trans) + em  and
  first-index argmax tie-breaking (via reverse-index code + max-reduce).
  Backpointers stored on-chip (uint8). Backtracking uses a one-hot
  select-and-reduce per step, entirely on-chip.
"""

import os
import numpy as np

S = 70
T = 37
B = 32768
NCORES = 8
BC = B // NCORES          # 4096 batch rows per core
NT = BC // 128            # 32 partition tiles per core
G = 4                     # tiles per instruction group
NG = NT // G              # groups per core

_PROGRAM_CACHE = {}
_VITERBI_OP = None
_VITERBI_OP2 = None
_VITERBI_BT = None


def _register_viterbi_bt():
    """Custom DVE op for backtracking: out = (within_page_idx == cur) ? hist : 0.

    in0 = hist [P, S, N] uint8, in1 = cur broadcast [P, S, N] (page-constant),
    s1 = N = 37 (compile-time).  within_page_idx = Idx - s1*SubIdx.
    """
    global _VITERBI_BT
    if _VITERBI_BT is not None:
        return _VITERBI_BT
    import concourse.dve_ops as dve_ops
    from concourse.dve_ops import DveOp, OPS, has_src1, lower
    from concourse.dve_spec import Spec, Src0, Src1, C1, Zero, select, eq, Idx, SubIdx
    from concourse.dve_uop import DveOpSpec

    body = select(eq(Idx - C1 * SubIdx, Src1), Src0, Zero)

    def _ref(in0, in1, s0, s1, imm2):
        assert in0.ndim == 3
        P, Sp, N = in0.shape
        k = np.arange(Sp * N, dtype=np.float64).reshape(Sp, N)
        sub = np.arange(Sp, dtype=np.float64)[:, None]
        wi = (k - s1 * sub).astype(np.float32)
        return np.where(
            wi[None] == in1.astype(np.float32), in0.astype(np.float32), np.float32(0.0)
        ).astype(np.float32)

    spec = Spec(body=body, reference=_ref)
    op = DveOp("VITERBI_BT", spec, subdim=True, uops_sha={})
    row = max(dve_ops._SUB_OPCODE_FOR_NAME.values()) + 1
    assert row < 0x20
    OPS.append(op)
    dve_ops._SUB_OPCODE_FOR_NAME[op.name] = row
    dve_ops.CUSTOM_DVE_SPECS[op.name] = op.spec
    for ver in ("v3", "v4"):
        try:
            compiled = DveOpSpec(
                name=op.name, opcode=row, uops=lower(spec, ver=ver),
                rd1_en=has_src1(spec),
            )
            op.uops_sha[ver] = compiled.sha(ver)
        except Exception:
            pass
    _VITERBI_BT = op
    return op


def _register_viterbi_op2():
    """Custom DVE op: running max-scan of page-offset tie codes.

    val_k = (w_k == best_page) ? 37*page + (37 - within_idx) : 0
          = (w_k == best_page) ? (s0 - Idx) + s1*SubIdx : 0   with s0=37, s1=74
    out_k = running max of val over the stream.

    Since code = 37 - within_idx is in [1, 37] and every page contains its own
    max, page p's matched values (37p+1 .. 37p+37) strictly dominate all
    earlier pages' values (<= 37p).  Reading out at each page's last element
    gives 37*page + (37 - first_argmax_idx) exactly (first-index tie-break via
    max over descending codes).
    """
    global _VITERBI_OP2
    if _VITERBI_OP2 is not None:
        return _VITERBI_OP2
    import concourse.dve_ops as dve_ops
    from concourse.dve_ops import DveOp, OPS, has_src1, lower
    from concourse.dve_spec import (
        Spec, Src0, Src1, C0, C1, Zero, select, eq, Idx, SubIdx, AluOp, scan,
    )
    from concourse.dve_uop import DveOpSpec

    val = select(eq(Src0, Src1), (C0 - Idx) + C1 * SubIdx, Zero)
    body = scan(AluOp.MAX, val)

    def _ref(in0, in1, s0, s1, imm2):
        assert in0.ndim == 3
        P, Sp, N = in0.shape
        k = np.arange(Sp * N, dtype=np.float64).reshape(Sp, N)
        sub = np.arange(Sp, dtype=np.float64)[:, None]
        code = ((s0 - k) + s1 * sub).astype(np.float32)
        v = np.where(in0 == in1, code[None], np.float32(0.0)).astype(np.float32)
        return np.maximum.accumulate(v.reshape(P, Sp * N), axis=1).reshape(P, Sp, N)

    spec = Spec(body=body, reference=_ref)
    op = DveOp("VITERBI_CODE2", spec, subdim=True, uops_sha={})
    row = max(dve_ops._SUB_OPCODE_FOR_NAME.values()) + 1
    assert row < 0x20
    OPS.append(op)
    dve_ops._SUB_OPCODE_FOR_NAME[op.name] = row
    dve_ops.CUSTOM_DVE_SPECS[op.name] = op.spec
    for ver in ("v3", "v4"):
        try:
            compiled = DveOpSpec(
                name=op.name, opcode=row, uops=lower(spec, ver=ver),
                rd1_en=has_src1(spec),
            )
            op.uops_sha[ver] = compiled.sha(ver)
        except Exception:
            pass
    _VITERBI_OP2 = op
    return op


def _register_viterbi_op():
    """Custom DVE op: code = (w == best) ? (37 - within_page_idx) : 0.

    in0 = w [P, S, N], in1 = best broadcast [P, S, N] (page-constant),
    s0 = 37.0, s1 = 37.0 (compile-time).  within_page_idx = Idx - N*SubIdx,
    so the value is (s0 - Idx) + s1*SubIdx with s1 = N = 37.
    """
    global _VITERBI_OP
    if _VITERBI_OP is not None:
        return _VITERBI_OP
    import concourse.dve_ops as dve_ops
    from concourse.dve_ops import DveOp, OPS, has_src1, lower
    from concourse.dve_spec import Spec, Src0, Src1, C0, C1, Zero, select, eq, Idx, SubIdx
    from concourse.dve_uop import DveOpSpec

    body = select(eq(Src0, Src1), (C0 - Idx) + C1 * SubIdx, Zero)

    def _ref(in0, in1, s0, s1, imm2):
        assert in0.ndim == 3
        P, Sp, N = in0.shape
        k = np.arange(Sp * N, dtype=np.float64).reshape(Sp, N)
        sub = np.arange(Sp, dtype=np.float64)[:, None]
        code = ((s0 - k) + s1 * sub).astype(np.float32)
        return np.where(in0 == in1, code[None], np.float32(0.0)).astype(np.float32)

    spec = Spec(body=body, reference=_ref)
    op = DveOp("VITERBI_CODE", spec, subdim=True, uops_sha={})
    # runtime registration: opcode row + sha pinning
    row = max(dve_ops._SUB_OPCODE_FOR_NAME.values()) + 1
    assert row < 0x20
    OPS.append(op)
    dve_ops._SUB_OPCODE_FOR_NAME[op.name] = row
    dve_ops.CUSTOM_DVE_SPECS[op.name] = op.spec
    for ver in ("v3", "v4"):
        try:
            compiled = DveOpSpec(
                name=op.name, opcode=row, uops=lower(spec, ver=ver),
                rd1_en=has_src1(spec),
            )
            op.uops_sha[ver] = compiled.sha(ver)
        except Exception:
            pass
    _VITERBI_OP = op
    return op


def _build_program(s_len=S, ng=NG, g=G):
    import concourse.bass as bass
    import concourse.tile as tile
    from concourse import bacc, mybir

    f32 = mybir.dt.float32
    u8 = mybir.dt.uint8
    i32 = mybir.dt.int32
    Alu = mybir.AluOpType
    X = mybir.AxisListType.X

    nt = ng * g              # partition tiles
    bc = nt * 128            # batch rows this core
    TT2 = T * T              # 1369
    NPG = g * T              # pages per instruction group (148)
    NC2 = TT2 + 3 * T + NPG  # consts packed: transflat, revi, iota, endt, pat37

    vop = _register_viterbi_op()
    btop = _register_viterbi_bt()
    nc = bacc.Bacc()

    # em layout host-prepped: [s, group, 128, g*T]
    em_d = nc.declare_dram_parameter("em", [s_len, ng, 128, g * T], f32, isOutput=False)
    # score0 layout host-prepped: [128, nt*T]
    score0_d = nc.declare_dram_parameter("score0", [128, nt * T], f32, isOutput=False)
    consts_d = nc.declare_dram_parameter("consts", [128, NC2], f32, isOutput=False)
    s_out = s_len
    tags_d = nc.declare_dram_parameter("tags", [bc, s_out], i32, isOutput=True)

    with tile.TileContext(nc) as tc:
        with (
            tc.tile_pool(name="const", bufs=1) as cpool,
            tc.tile_pool(name="em", bufs=2) as empool,
            tc.tile_pool(name="score", bufs=1) as spool,
            tc.tile_pool(name="zbuf", bufs=2) as zpool,
            tc.tile_pool(name="wbuf", bufs=1) as wpool,
            tc.tile_pool(name="hist", bufs=1) as hpool,
            tc.tile_pool(name="bt", bufs=2) as btpool,
            tc.tile_pool(name="small", bufs=2) as smpool,
        ):
            # ---- constants (single DMA) ----
            consts = cpool.tile([128, NC2], f32)
            nc.sync.dma_start(consts[:], consts_d[:])
            transflat = consts[:, 0:TT2]
            revi = consts[:, TT2 : TT2 + T]
            iota = consts[:, TT2 + T : TT2 + 2 * T]
            endt = consts[:, TT2 + 2 * T : TT2 + 3 * T]
            pat37 = consts[:, TT2 + 3 * T : TT2 + 3 * T + NPG]

            # ---- persistent state ----
            hist = hpool.tile([128, (s_len - 1) * nt * T], u8)
            # tags staged as uint8 (values 0..36); converted to int32 at the
            # end reusing a z-pool slot
            tags_sb = hpool.tile([128, nt * s_out], u8, tag="tags_sb")
            # 40-padded bf16 code buffer: pages of 37 codes + 3 zero holes so
            # the pairwise-max tree slices stay 4B-aligned (bf16 2x mode)
            bf16 = mybir.dt.bfloat16
            NPAD = 40
            cpad = hpool.tile([128, g * T * NPAD], bf16, tag="cpad")
            nc.vector.memset(cpad[:], 0.0)
            m1 = hpool.tile([128, g * T * 20], bf16, tag="m1")
            # m2 padded 10->12 wide so the next pair-max level stays 4B-aligned
            m2 = hpool.tile([128, g * T * 12], bf16, tag="m2")
            nc.vector.memset(m2[:], 0.0)
            m3 = hpool.tile([128, g * T * 6], bf16, tag="m3")

            # score ping-pong buffers [128, nt*T]
            score_a = spool.tile([128, nt * T], f32, tag="score_a")
            score_b = spool.tile([128, nt * T], f32, tag="score_b")
            nc.sync.dma_start(score_a[:], score0_d[:])
            cur_score, nxt_score = score_a, score_b

            tf_b = (
                transflat.rearrange("p (j i) -> p j i", i=T)
                .unsqueeze(1)
                .broadcast_to([128, g, T, T])
            )
            revi_b = revi.unsqueeze(1).unsqueeze(1).broadcast_to([128, g, T, T])

            # ---- forward scan ----
            for s in range(1, s_len):
                for gi in range(ng):
                    em_t = empool.tile([128, g * T], f32, tag="em")
                    nc.sync.dma_start(em_t[:], em_d[s, gi])

                    sc3 = cur_score[:, gi * g * T : (gi + 1) * g * T].rearrange(
                        "p (tt i) -> p tt i", i=T
                    )
                    sc_b = sc3.unsqueeze(2).broadcast_to([128, g, T, T])
                    em_b = (
                        em_t[:]
                        .rearrange("p (tt j) -> p tt j", j=T)
                        .unsqueeze(3)
                        .broadcast_to([128, g, T, T])
                    )

                    zt = zpool.tile([128, g * TT2], f32, tag="z")
                    z4 = zt[:].rearrange("p (tt j i) -> p tt j i", j=T, i=T)
                    # z = score + trans   (score[b,i] + trans[i,j] at [j,i])
                    # on GPSIMD to overlap with the DVE passes
                    nc.gpsimd.tensor_tensor(z4, sc_b, tf_b, Alu.add)

                    wt = wpool.tile([128, g * TT2], f32, tag="w")
                    w4 = wt[:].rearrange("p (tt j i) -> p tt j i", j=T, i=T)
                    # w = z + em[b,j]  (4 of 8 groups on GPSIMD; DVE fixed work
                    # dropped to ~15.2k cyc/group with the bf16 tree, so it
                    # takes back one w-add)
                    weng = nc.vector if gi in (1, 3, 4, 6) else nc.gpsimd
                    weng.tensor_tensor(w4, z4, em_b, Alu.add)

                    # new score = max_i w
                    ns3 = nxt_score[:, gi * g * T : (gi + 1) * g * T].rearrange(
                        "p (tt j) -> p tt j", j=T
                    )
                    nc.vector.tensor_reduce(ns3, w4, X, Alu.max)

                    # code = (w == best) ? (37 - i) : 0   (fused custom op,
                    # emitted as bf16 into the 40-padded page layout)
                    w3 = wt[:].rearrange("p (sj i) -> p sj i", i=T)
                    ns_pb = (
                        nxt_score[:, gi * g * T : (gi + 1) * g * T]
                        .unsqueeze(2)
                        .broadcast_to([128, NPG, T])
                    )
                    cp3 = cpad[:].rearrange("p (sj i) -> p sj i", i=NPAD)
                    nc.vector._custom_dve(
                        vop, out=cp3[:, :, 0:T], in0=w3, in1=ns_pb,
                        s0=float(T), s1=float(T),
                    )

                    # hist codes = max_i code -> uint8, via two bf16 2x-mode
                    # pairwise max levels (40->20->10) + small 1x reduce
                    m1_3 = m1[:].rearrange("p (sj i) -> p sj i", i=20)
                    nc.vector.tensor_tensor(
                        m1_3, cp3[:, :, 0:20], cp3[:, :, 20:40], Alu.max
                    )
                    m2_3 = m2[:].rearrange("p (sj i) -> p sj i", i=12)
                    nc.vector.tensor_tensor(
                        m2_3[:, :, 0:10], m1_3[:, :, 0:10], m1_3[:, :, 10:20], Alu.max
                    )
                    m3_3 = m3[:].rearrange("p (sj i) -> p sj i", i=6)
                    nc.vector.tensor_tensor(
                        m3_3, m2_3[:, :, 0:6], m2_3[:, :, 6:12], Alu.max
                    )
                    hoff = ((s - 1) * nt + gi * g) * T
                    hslice = hist[:, hoff : hoff + g * T]
                    nc.vector.tensor_reduce(hslice, m3_3, X, Alu.max)

                cur_score, nxt_score = nxt_score, cur_score

            # ---- final argmax over tags (score + end_transitions) ----
            cur = btpool.tile([128, nt], f32, tag="cur")
            endt_b = endt.unsqueeze(1).broadcast_to([128, g, T])
            revi_b2 = revi.unsqueeze(1).broadcast_to([128, g, T])
            for gi in range(ng):
                sc3 = cur_score[:, gi * g * T : (gi + 1) * g * T].rearrange(
                    "p (tt j) -> p tt j", j=T
                )
                se = smpool.tile([128, g * T], f32, tag="se")
                se3 = se[:].rearrange("p (tt j) -> p tt j", j=T)
                nc.vector.tensor_tensor(se3, sc3, endt_b, Alu.add)
                b1 = smpool.tile([128, g], f32, tag="b1")
                nc.vector.tensor_reduce(b1[:], se3, X, Alu.max)
                b1_b = b1[:].unsqueeze(2).broadcast_to([128, g, T])
                eqf = smpool.tile([128, g * T], f32, tag="eqf")
                eqf3 = eqf[:].rearrange("p (tt j) -> p tt j", j=T)
                nc.vector.tensor_tensor(eqf3, se3, b1_b, Alu.is_equal)
                nc.vector.tensor_tensor(eqf3, eqf3, revi_b2, Alu.mult)
                codef = smpool.tile([128, g], f32, tag="codef")
                nc.vector.tensor_reduce(codef[:], eqf3, X, Alu.max)
                # cur = 37 - code
                nc.vector.tensor_scalar(
                    cur[:, gi * g : (gi + 1) * g], codef[:], -1.0, float(T), Alu.mult, Alu.add
                )

            # tags column s_len-1
            tags3 = tags_sb[:].rearrange("p (tt s) -> p tt s", s=s_out)
            nc.vector.tensor_copy(tags3[:, :, s_len - 1], cur[:])

            # ---- backtracking ----
            for s in range(s_len - 1, 0, -1):
                cur_b = cur[:].unsqueeze(2).broadcast_to([128, nt, T])
                eqb = btpool.tile([128, nt * T], f32, tag="eqb")
                eqb3 = eqb[:].rearrange("p (tt i) -> p tt i", i=T)
                hoff = (s - 1) * nt * T
                h3 = hist[:, hoff : hoff + nt * T].rearrange("p (tt i) -> p tt i", i=T)
                # eqb = (within_idx == cur) ? hist : 0   (fused custom op)
                nc.vector._custom_dve(
                    btop, out=eqb3, in0=h3, in1=cur_b, s0=0.0, s1=float(T)
                )
                pcode = btpool.tile([128, nt], f32, tag="pcode")
                nc.vector.tensor_reduce(pcode[:], eqb3, X, Alu.max)
                ncur = btpool.tile([128, nt], f32, tag="cur")
                nc.vector.tensor_scalar(ncur[:], pcode[:], -1.0, float(T), Alu.mult, Alu.add)
                cur = ncur
                nc.vector.tensor_copy(tags3[:, :, s - 1], cur[:])

            # ---- convert tags to int32 (z-pool slot is free now) and DMA ----
            tags_i32 = zpool.tile([128, nt * s_out], i32, tag="z")
            nc.vector.tensor_copy(tags_i32[:], tags_sb[:])
            nc.sync.dma_start(
                tags_d[:].rearrange("(tt p) s -> p tt s", p=128),
                tags_i32[:].rearrange("p (tt s) -> p tt s", s=s_out),
            )

    nc.finalize()
    return nc


def _host_prep(emissions, mask, start_transitions, end_transitions, transitions,
               s_len=S, ng=NG, g=G, ncores=NCORES):
    nt = ng * g
    bc = nt * 128
    em = np.ascontiguousarray(np.asarray(emissions, dtype=np.float32))
    start = np.asarray(start_transitions, dtype=np.float32)
    end = np.asarray(end_transitions, dtype=np.float32)
    trans = np.asarray(transitions, dtype=np.float32)

    score0 = (start[None, :] + em[0]).astype(np.float32)  # [B, T]

    # per-core reorders
    b_total = em.shape[1]
    em_r = em.reshape(s_len, b_total // bc, ng, g, 128, T)
    # -> [core][s, ng, 128, g*T]
    em_cores = [
        np.ascontiguousarray(em_r[:, c].transpose(0, 1, 3, 2, 4).reshape(s_len, ng, 128, g * T))
        for c in range(b_total // bc)
    ]
    s0_r = score0.reshape(b_total // bc, nt, 128, T)
    score0_cores = [
        np.ascontiguousarray(s0_r[c].transpose(1, 0, 2).reshape(128, nt * T))
        for c in range(b_total // bc)
    ]

    # consts: transflat (j-major: trans[i,j] at j*T+i), revi, iota, endt
    transflat = np.ascontiguousarray(trans.T).reshape(T * T)
    revi = (T - np.arange(T)).astype(np.float32)
    iota = np.arange(T).astype(np.float32)
    pat37 = (T * np.arange(g * T)).astype(np.float32)
    consts = np.concatenate([transflat, revi, iota, end, pat37]).astype(np.float32)
    consts = np.broadcast_to(consts[None, :], (128, consts.size)).copy()
    return em_cores, score0_cores, consts


def kernel(emissions, mask, start_transitions, end_transitions, transitions):
    mask_np = np.asarray(mask)
    if not mask_np.all():
        return _numpy_reference(
            np.asarray(emissions, np.float32), mask_np,
            np.asarray(start_transitions, np.float32),
            np.asarray(end_transitions, np.float32),
            np.asarray(transitions, np.float32),
        )

    from concourse.bass_utils import run_bass_kernel_spmd

    # persistent jax compilation cache: skips the ~2min neuronxcc compile on
    # repeat runs (the bass program is serialized deterministically into HLO)
    try:
        import jax
        jax.config.update("jax_compilation_cache_dir", "/tmp/jax_neff_cache")
        jax.config.update("jax_persistent_cache_min_compile_time_secs", 5.0)
        jax.config.update("jax_persistent_cache_min_entry_size_bytes", 0)
    except Exception:
        pass

    em_cores, score0_cores, consts = _host_prep(
        emissions, mask, start_transitions, end_transitions, transitions
    )

    key = (S, NG, G)
    if key not in _PROGRAM_CACHE:
        _PROGRAM_CACHE[key] = _build_program(S, NG, G)
    nc = _PROGRAM_CACHE[key]

    in_maps = []
    for c in range(NCORES):
        in_maps.append(
            {"em": em_cores[c], "score0": score0_cores[c], "consts": consts}
        )

    res = run_bass_kernel_spmd(
        nc, in_maps, list(range(NCORES)), trace=bool(os.environ.get("VITERBI_TRACE"))
    )
    global LAST_EXEC_NS
    if res.exec_time_ns:
        LAST_EXEC_NS = res.exec_time_ns
    tags = np.concatenate([np.asarray(r["tags"]) for r in res.results], axis=0)
    return tags.astype(np.int32)


LAST_EXEC_NS = None


def _numpy_reference(em, mask, start, end, trans):
    S_, B_, T_ = em.shape
    score = (start[None, :] + em[0]).astype(np.float32)
    history = np.zeros((S_ - 1, B_, T_), dtype=np.int32)
    for s in range(1, S_):
        z = score[:, :, None] + trans[None, :, :]
        ns = z + em[s][:, None, :]
        idx = np.argmax(ns, axis=1).astype(np.int32)
        best = np.max(ns, axis=1)
        m = mask[s][:, None]
        score = np.where(m, best, score)
        history[s - 1] = idx
    score = score + end[None, :]
    seq_ends = mask.astype(np.int32).sum(0) - 1
    best_last = np.argmax(score, axis=1).astype(np.int32)
    barange = np.arange(B_)
    tags = np.zeros((S_, B_), dtype=np.int32)
    tags[S_ - 1] = best_last
    cur = best_last
    for i in range(S_ - 1, 0, -1):
        prev = history[i - 1][barange, cur]
        cur = np.where(i <= seq_ends, prev, cur).astype(np.int32)
        tags[i - 1] = cur
    tpos = np.arange(S_)[:, None]
    tags = np.where(tpos <= seq_ends[None, :], tags, -1)
    return tags.T.astype(np.int32)

